# revision 36
# baseline (speedup 1.0000x reference)
"""Trainium2 Bass kernel for nn_Net_76562087018570.

Computation (reference): per-column MinMax scale of a (4096, 8192) f32 matrix,
10 iterations of arr = arr*(1 - (arr - rowmean(arr))) (+0.001 on iter 0),
then inverse transform.

Strategy: shard rows across 8 cores (512 rows each).  Carry
sq_k = (arr_k - h_k)^2 with h_k = (1+mean_k)/2 so each iteration is a single
square pass with a fused row-sum:
    sq_{k+1} = (beta_k - sq_k)^2,  beta_k = rowsum(sq_k)/(2n) + (gamma_k-1)/2.
Each square pass is column-split between ACT (Square activation) and a custom
DVE op `sq(Src0-C0)` with fused accum (1 elem/cycle) so both engines run flat
out.  The whole data path is fp16 (~5.5e-3 rel err vs the 2e-2 gate): fp16
gives 2x/4x DVE modes and halves SBUF traffic; loads/stores cast fp32<->fp16
inside gpsimd (SWDGE) DMAs so no engine pass is spent on dtype conversion.
The per-column min/max -> AllReduce -> broadcast -> normalize front-end is
split into column halves and pipelined: the left half's collective runs while
the right half is still reducing, and the left normalization overlaps the
right collective.  Per-iteration scalar chains run per-tile (u/beta on DVE,
the rest on GPSIMD) to avoid cross-tile barriers; the last iteration runs
fully on ACT in half tiles while DVE does the inverse transform behind it,
with cast-and-store DMAs draining per half tile.
"""

import os
import numpy as np

R = 512          # rows per core
N = 8192         # columns
H = N // 2
NT = 4           # (128,N) tiles per core
NCORES = 8
NITERS = 10
# per-iteration DVE column counts (rest goes to ACT); last round is ACT-only
# so DVE can run the inverse transform behind it
DCOLS = [3584, 3584, 3584, 3584, 3584, 3584, 3584, 3584, 0]

_cache = {}
LAST_RESULT = None


def _register_custom_ops():
    from concourse import dve_ops
    from concourse.dve_spec import Spec, Src0, C0, sq, lower, AluOp
    from concourse.dve_uop import DveOpSpec

    def make(name, spec):
        for op in dve_ops.OPS:
            if op.name == name:
                return op
        shas = {}
        for ver in ("v3", "v4"):
            tmp = DveOpSpec(name=name, opcode=0, uops=lower(spec, ver=ver),
                            rd1_en=dve_ops.has_src1(spec))
            shas[ver] = tmp.sha(ver)
        op = dve_ops.DveOp(name, spec, subdim=False, uops_sha=shas)
        dve_ops.OPS.append(op)
        dve_ops.CUSTOM_DVE_SPECS[op.name] = op.spec
        dve_ops._SUB_OPCODE_FOR_NAME[op.name] = (
            dve_ops._CUSTOM_DVE_ROW_BASE + len(dve_ops.OPS) - 1)
        assert dve_ops._SUB_OPCODE_FOR_NAME[op.name] < 0x20
        return op

    return make("ANT_ITER_SQ", Spec(
        body=sq(Src0 - C0), accum=AluOp.ADD,
        reference=lambda in0, s0: (in0 - s0) ** 2,
    ))


def _build():
    import concourse.bacc as bacc
    import concourse.tile as tile
    from concourse import mybir, masks

    ITER_OP = _register_custom_ops()

    f32 = mybir.dt.float32
    f16 = mybir.dt.float16
    A = mybir.AluOpType
    AF = mybir.ActivationFunctionType
    AX = mybir.AxisListType

    nc = bacc.Bacc(trn_type="TRN2", num_devices=NCORES)
    xs = nc.dram_tensor("xs", [R, N], f32, kind="ExternalInput")
    out = nc.dram_tensor("out", [R, N], f32, kind="ExternalOutput")
    xv = xs.ap().rearrange("(t p) n -> t p n", p=128)
    ov = out.ap().rearrange("(t p) n -> t p n", p=128)

    with tile.TileContext(nc) as tc:
        with tc.tile_pool(name="state", bufs=1) as st, \
             tc.tile_pool(name="mm", bufs=1) as mm, \
             tc.tile_pool(name="bc", bufs=1) as bc, \
             tc.tile_pool(name="small", bufs=8) as small, \
             tc.tile_pool(name="psum", bufs=2, space="PSUM") as psum, \
             tc.tile_pool(name="psq", bufs=1, space="PSUM") as psq, \
             tc.tile_pool(name="dram", bufs=1, space="DRAM") as dram:

            ident16 = mm.tile([128, 128], f16)
            masks.make_identity(nc, ident16[:])
            ones32 = mm.tile([128, 1], f32)
            nc.vector.memset(ones32[:], 1.0)
            ones_row = mm.tile([1, 128], f32)
            nc.vector.memset(ones_row[:], 1.0)

            # ---- load (cast f32->f16 in the DMA), left halves first ----
            W = [st.tile([128, N], f16, name=f"w{t}") for t in range(NT)]
            for t in range(NT):
                nc.gpsimd.dma_start(W[t][:, 0:H], xv[t][:, 0:H])
            for t in range(NT):
                nc.gpsimd.dma_start(W[t][:, H:N], xv[t][:, H:N])

            # ---- per-half pipeline state ----
            mrun = mm.tile([128, N], f16, name="mrun")
            xrun = mm.tile([128, N], f16, name="xrun")
            rb = bc.tile([128, N], f16, name="rb")
            qb = bc.tile([128, N], f16, name="qb")
            safeb = bc.tile([128, N], f16, name="safeb")
            mnb = bc.tile([128, N], f16, name="mnb")
            q_d = dram.tile([1, N], f16)
            rinv_d = dram.tile([1, N], f16)
            safe_d = dram.tile([1, N], f16)
            mn_d = dram.tile([1, N], f16)
            qvs = {}
            aws = {"L": [], "R": []}

            for side, (s, e) in (("L", (0, H)), ("R", (H, N))):
                nb = (e - s) // 128          # 32 col-blocks per half
                # min/max combine tree on this half
                nc.vector.tensor_tensor(mrun[:, s:e], W[0][:, s:e],
                                        W[1][:, s:e], op=A.min)
                nc.vector.tensor_tensor(xrun[:, s:e], W[0][:, s:e],
                                        W[1][:, s:e], op=A.max)
                for t in (2, 3):
                    nc.vector.tensor_tensor(mrun[:, s:e], mrun[:, s:e],
                                            W[t][:, s:e], op=A.min)
                    nc.vector.tensor_tensor(xrun[:, s:e], xrun[:, s:e],
                                            W[t][:, s:e], op=A.max)
                # transpose + reduce -> per-column min/max (partition-major)
                vmin = mm.tile([128, 32], f32, name=f"vmin{side}")
                vmax = mm.tile([128, 32], f32, name=f"vmax{side}")
                for src, dst, op in ((mrun, vmin, A.min), (xrun, vmax, A.max)):
                    for g in range(4):       # 4 groups of 8 col-blocks
                        pt = psum.tile([128, 1024], f16, name="pt")
                        for j in range(8):
                            cb = g * 8 + j
                            nc.tensor.transpose(
                                pt[:, j * 128:(j + 1) * 128],
                                src[:, s + cb * 128:s + (cb + 1) * 128],
                                ident16[:])
                        nc.vector.tensor_reduce(
                            out=dst[:, g * 8:(g + 1) * 8],
                            in_=pt[:].rearrange("p (c x) -> p c x", c=8),
                            axis=AX.X, op=op)
                # pack [max | -min] (128,64) f16, AllReduce(max) across cores
                vpair = mm.tile([128, 64], f16, name=f"vp{side}")
                nc.vector.tensor_scalar(out=vpair[:, 32:64], in0=vmin[:],
                                        scalar1=-1.0, scalar2=None, op0=A.mult)
                nc.vector.tensor_copy(vpair[:, 0:32], vmax[:])
                cc_in = dram.tile([128, 64], f16)
                cc_out = dram.tile([128, 64], f16, addr_space="Shared")
                nc.sync.dma_start(cc_in[:], vpair[:])
                nc.gpsimd.collective_compute(
                    "AllReduce", A.max,
                    replica_groups=[list(range(NCORES))],
                    ins=[cc_in[:]], outs=[cc_out[:]],
                )
                gpair = mm.tile([128, 64], f16, name=f"gp{side}")
                with tc.high_priority():
                    nc.sync.dma_start(gpair[:], cc_out[:])

                # column stats for this half (f32 smalls on f16 inputs)
                mnv = mm.tile([128, 32], f32, name=f"mn{side}")
                nc.vector.tensor_scalar(out=mnv[:], in0=gpair[:, 32:64],
                                        scalar1=-1.0, scalar2=None, op0=A.mult)
                rng = mm.tile([128, 32], f32, name=f"rg{side}")
                nc.vector.tensor_tensor(rng[:], gpair[:, 0:32], mnv[:],
                                        op=A.subtract)
                eq0 = mm.tile([128, 32], f32, name=f"eq{side}")
                nc.vector.tensor_scalar(out=eq0[:], in0=rng[:],
                                        scalar1=0.0, scalar2=None,
                                        op0=A.is_equal)
                safe = mm.tile([128, 32], f32, name=f"sf{side}")
                nc.vector.tensor_tensor(safe[:], rng[:], eq0[:], op=A.add)
                rinv = mm.tile([128, 32], f32, name=f"ri{side}")
                nc.vector.reciprocal(rinv[:], safe[:])
                qv = mm.tile([128, 32], f32, name=f"qv{side}")
                nc.vector.scalar_tensor_tensor(
                    out=qv[:], in0=mnv[:], scalar=-1.0, in1=rinv[:],
                    op0=A.mult, op1=A.mult)
                qvs[side] = qv

                # relayout q,rinv -> (1,half) f16 rows, broadcast rb/qb
                pk = mm.tile([128, 64], f16, name=f"pk{side}")
                nc.vector.tensor_copy(pk[:, 0:32], qv[:])
                nc.vector.tensor_copy(pk[:, 32:64], rinv[:])
                tp_ = psq.tile([64, 128], f16, name=f"tp{side}")
                nc.tensor.transpose(tp_[:], pk[:], ident16[:])
                tps = mm.tile([64, 128], f16, name=f"tq{side}")
                nc.scalar.copy(tps[:], tp_[:])
                nc.sync.dma_start(
                    q_d[:, s:e].rearrange("o (f p) -> (o f) p", p=128),
                    tps[0:32, :])
                nc.sync.dma_start(
                    rinv_d[:, s:e].rearrange("o (f p) -> (o f) p", p=128),
                    tps[32:64, :])
                for cs in (s, s + (e - s) // 2):
                    ce = cs + (e - s) // 2
                    nc.sync.dma_start(
                        rb[:, cs:ce], rinv_d[:, cs:ce].to_broadcast(
                            (128, ce - cs)))
                for cs in (s, s + (e - s) // 2):
                    ce = cs + (e - s) // 2
                    nc.sync.dma_start(
                        qb[:, cs:ce], q_d[:, cs:ce].to_broadcast(
                            (128, ce - cs)))

                # safe/mn relayout (only needed for the inverse transform)
                pkb = mm.tile([128, 64], f16, name=f"pb{side}")
                nc.vector.tensor_copy(pkb[:, 0:32], safe[:])
                nc.vector.tensor_copy(pkb[:, 32:64], mnv[:])
                tpb = psq.tile([64, 128], f16, name=f"tb{side}")
                nc.tensor.transpose(tpb[:], pkb[:], ident16[:])
                tbs = mm.tile([64, 128], f16, name=f"tr{side}")
                nc.scalar.copy(tbs[:], tpb[:])
                nc.sync.dma_start(
                    safe_d[:, s:e].rearrange("o (f p) -> (o f) p", p=128),
                    tbs[0:32, :])
                nc.sync.dma_start(
                    mn_d[:, s:e].rearrange("o (f p) -> (o f) p", p=128),
                    tbs[32:64, :])

                # startup P1 on this half: w = a*rinv in place, fused row-sum
                for t in range(NT):
                    aw = small.tile([128, 1], f32, name=f"aw{side}{t}")
                    nc.vector.scalar_tensor_tensor(
                        out=W[t][:, s:e], in0=W[t][:, s:e], scalar=0.0,
                        in1=rb[:, s:e], op0=A.bypass, op1=A.mult,
                        accum_out=aw[:])
                    aws[side].append(aw)

            # Q = sum(q) via PE contraction + broadcast back
            qsum = mm.tile([128, 1], f32)
            qr_l = mm.tile([128, 1], f32, name="qrl")
            qr_r = mm.tile([128, 1], f32, name="qrr")
            nc.vector.tensor_reduce(out=qr_l[:], in_=qvs["L"][:], axis=AX.X,
                                    op=A.add)
            nc.vector.tensor_reduce(out=qr_r[:], in_=qvs["R"][:], axis=AX.X,
                                    op=A.add)
            nc.vector.tensor_tensor(qsum[:], qr_l[:], qr_r[:], op=A.add)
            pq1 = psq.tile([1, 1], f32, name="pq1")
            nc.tensor.matmul(pq1[:], qsum[:], ones32[:])
            sq1 = mm.tile([1, 1], f32)
            nc.scalar.copy(sq1[:], pq1[:])
            pq2 = psq.tile([128, 1], f32, name="pq2")
            nc.tensor.matmul(pq2[:], ones_row[:], sq1[:])
            Qs = mm.tile([128, 1], f32)
            nc.scalar.copy(Qs[:], pq2[:])
            Qh = mm.tile([128, 1], f32)
            nc.vector.tensor_scalar(out=Qh[:], in0=Qs[:],
                                    scalar1=0.5 / N, scalar2=0.5,
                                    op0=A.mult, op1=A.add)

            # ---- startup finish per tile: h0, P2a (+q), P2b square ----
            gams, ds = [], []
            accs_a, accs_b = [], []
            for t in range(NT):
                sw = small.tile([128, 1], f32, name=f"sw{t}")
                nc.vector.tensor_tensor(sw[:], aws["L"][t][:], aws["R"][t][:],
                                        op=A.add)
                h0 = small.tile([128, 1], f32, name=f"h0_{t}")
                nc.vector.tensor_scalar(out=h0[:], in0=sw[:],
                                        scalar1=0.5 / N, scalar2=Qh[:],
                                        op0=A.mult, op1=A.add)
                nc.vector.tensor_tensor(W[t][:], W[t][:], qb[:], op=A.add)
                acc_l = small.tile([128, 1], f32, name=f"sl{t}")
                acc_r = small.tile([128, 1], f32, name=f"sr{t}")
                nc.scalar.activation(
                    W[t][:, 0:H], W[t][:, 0:H], AF.Square,
                    bias=h0[:], scale=-1.0, accum_out=acc_l[:])
                nc.scalar.activation(
                    W[t][:, H:N], W[t][:, H:N], AF.Square,
                    bias=h0[:], scale=-1.0, accum_out=acc_r[:])
                accs_a.append(acc_l); accs_b.append(acc_r)
                gam = small.tile([128, 1], f32, name=f"g{t}")
                nc.gpsimd.tensor_scalar(out=gam[:], in0=h0[:],
                                        scalar1=h0[:], scalar2=0.001,
                                        op0=A.mult, op1=A.add)
                d = small.tile([128, 1], f32, name=f"d{t}")
                nc.gpsimd.tensor_scalar(out=d[:], in0=gam[:],
                                        scalar1=0.5, scalar2=-0.5,
                                        op0=A.mult, op1=A.add)
                gams.append(gam); ds.append(d)

            # ---- iterations k = 1..9 ----
            for k in range(1, NITERS):
                if k == 3:
                    # safe/mn broadcasts (needed only by the inverse
                    # transform); emitted here so they stay off the DMA
                    # queues while rb/qb gate the startup
                    for cs in range(0, N, N // 4):
                        ce = cs + N // 4
                        nc.sync.dma_start(
                            safeb[:, cs:ce],
                            safe_d[:, cs:ce].to_broadcast((128, ce - cs)))
                        nc.sync.dma_start(
                            mnb[:, cs:ce],
                            mn_d[:, cs:ce].to_broadcast((128, ce - cs)))
                dc = DCOLS[k - 1]
                ac = N - dc
                betas = []
                for t in range(NT):
                    u = small.tile([128, 1], f32, name=f"u{t}")
                    nc.vector.tensor_tensor(u[:], accs_a[t][:],
                                            accs_b[t][:], op=A.add)
                    beta = small.tile([128, 1], f32, name=f"b{t}")
                    nc.vector.tensor_scalar(out=beta[:], in0=u[:],
                                            scalar1=0.5 / N, scalar2=ds[t][:],
                                            op0=A.mult, op1=A.add)
                    betas.append(beta)
                ngams, nds = [], []
                accs_a, accs_b = [], []
                for t in range(NT):
                    beta = betas[t]
                    last = (k == NITERS - 1)
                    if last:
                        # round 9: all on ACT in halves (no accums needed);
                        # DVE runs the inverse transform right behind
                        nc.scalar.activation(
                            W[t][:, 0:H], W[t][:, 0:H], AF.Square,
                            bias=beta[:], scale=-1.0)
                        nc.scalar.activation(
                            W[t][:, H:N], W[t][:, H:N], AF.Square,
                            bias=beta[:], scale=-1.0)
                    else:
                        acc_a = small.tile([128, 1], f32, name=f"aa{t}")
                        acc_b = small.tile([128, 1], f32, name=f"ab{t}")
                        nc.scalar.activation(
                            W[t][:, 0:ac], W[t][:, 0:ac], AF.Square,
                            bias=beta[:], scale=-1.0, accum_out=acc_a[:])
                        nc.vector._custom_dve(
                            ITER_OP, out=W[t][:, ac:N], accum_out=acc_b[:],
                            in0=W[t][:, ac:N], s0=beta[:])
                        accs_a.append(acc_a); accs_b.append(acc_b)
                    h = small.tile([128, 1], f32, name=f"h{t}")
                    nc.gpsimd.tensor_scalar(out=h[:], in0=beta[:],
                                            scalar1=gams[t][:], scalar2=-1.0,
                                            op0=A.subtract, op1=A.mult)
                    gam = small.tile([128, 1], f32, name=f"g{t}")
                    nc.gpsimd.tensor_scalar(out=gam[:], in0=h[:],
                                            scalar1=h[:], scalar2=None,
                                            op0=A.mult)
                    ngams.append(gam)
                    if not last:
                        d = small.tile([128, 1], f32, name=f"d{t}")
                        nc.gpsimd.tensor_scalar(out=d[:], in0=gam[:],
                                                scalar1=0.5, scalar2=-0.5,
                                                op0=A.mult, op1=A.add)
                        nds.append(d)
                    else:
                        # final: out = mnb - (sq9 - gamma9)*safe, in place,
                        # per half tile so store DMAs drain early
                        for fs, fe in ((0, H), (H, N)):
                            nc.vector.tensor_scalar(
                                out=W[t][:, fs:fe], in0=W[t][:, fs:fe],
                                scalar1=gam[:], scalar2=None, op0=A.subtract)
                            nc.vector.tensor_tensor(
                                W[t][:, fs:fe], W[t][:, fs:fe],
                                safeb[:, fs:fe], op=A.mult)
                            nc.vector.tensor_tensor(
                                W[t][:, fs:fe], mnb[:, fs:fe], W[t][:, fs:fe],
                                op=A.subtract)
                            nc.gpsimd.dma_start(ov[t][:, fs:fe],
                                                W[t][:, fs:fe])
                gams, ds = ngams, nds

    if not nc.is_finalized():
        nc.finalize()
    return nc


def _get_nc():
    if "nc" not in _cache:
        _cache["nc"] = _build()
    return _cache["nc"]


def kernel(x):
    global LAST_RESULT
    from concourse.bass_utils import run_bass_kernel_spmd

    x = np.ascontiguousarray(np.asarray(x), dtype=np.float32)
    a = x.reshape(NCORES * R, N)
    nc = _get_nc()
    in_maps = [{"xs": np.ascontiguousarray(a[c * R:(c + 1) * R])}
               for c in range(NCORES)]
    res = run_bass_kernel_spmd(
        nc, in_maps, core_ids=list(range(NCORES)),
        trace=bool(int(os.environ.get("KBENCH_TRACE", "0"))),
    )
    LAST_RESULT = res
    full = np.concatenate([res.results[c]["out"] for c in range(NCORES)], axis=0)
    return full.reshape(1, NCORES * R, N).astype(np.float32)
